# revision 6
# baseline (speedup 1.0000x reference)
"""Trainium2 Bass kernel for nn_DecoderRNN (LSTM decoder with tag-conditioned
inputs, packed-sequence output projection).

Strategy (8 NeuronCores, SPMD single program):
  - gx (input projection) for steps 0-3 computed replicated on every core so
    the scan starts ~50us in, without waiting for the first AllGather.
    Steps 4-31 are step-sharded: core c computes steps {4+c, 12+c, 20+c,
    28+(c%4)} and four pipelined AllGathers distribute them.  gx for steps
    1-3 / AG steps 5-7 is interleaved into scan steps 0-2 as TensorE filler.
  - LSTM recurrence: replicated full-batch on every core (per-step cross-core
    h exchange is latency-prohibitive).  Gate matmuls are emitted in
    (half, k-group) order; the cell tail runs on GpSimd/ACT/DVE and hides
    under TensorE streaming; vocab-projection units interleave as filler.
  - Output projection: vocab-sharded (V/8 per core); packed hidden states are
    kept resident in SBUF (no DRAM roundtrip, no epilogue reload stalls).
Compute dtype: fp16 operands into the PE (full rate), fp32 accumulation.
"""

import sys

sys.path.insert(0, "/opt/trn_rl_repo")

import numpy as np

import concourse.bass as bass
import concourse.mybir as mybir
import concourse.tile as tile
from concourse import bacc
from concourse.bass import ts
from concourse.bass_utils import run_bass_kernel_spmd
from concourse.masks import make_identity

B, L, E, H, V, TAG = 128, 31, 512, 1024, 30000, 512
T = L + 1
NC = 8
VS = V // NC          # vocab shard per core
G4 = 4 * H            # gate width
F16 = mybir.dt.float16
F32 = mybir.dt.float32
AF = mybir.ActivationFunctionType
NCH = (VS + 511) // 512  # projection vocab chunks per core
NGATHER = 8           # gather cols: steps 0,1,2,3 + 4 owned AG steps
GATE_FN = {0: AF.Sigmoid, 1: AF.Sigmoid, 2: AF.Tanh, 3: AF.Sigmoid}


def _build(n_t, off_t, p_pad):
    nc = bacc.Bacc(None, target_bir_lowering=False)

    emb_tab = nc.declare_dram_parameter("emb_tab", [V + B, E], F16, isOutput=False)
    idx_in = nc.declare_dram_parameter("idx", [B, NGATHER], mybir.dt.int32, isOutput=False)
    wihx = nc.declare_dram_parameter("wihx", [E, G4], F16, isOutput=False)
    wiht = nc.declare_dram_parameter("wiht", [5 * 128, G4], F16, isOutput=False)
    tags_t = nc.declare_dram_parameter("tags_t", [5 * 128, B], F16, isOutput=False)
    whh = nc.declare_dram_parameter("whh", [H, G4], F16, isOutput=False)
    wlin = nc.declare_dram_parameter("wlin", [H, VS], F16, isOutput=False)
    blin = nc.declare_dram_parameter("blin", [1, VS], F16, isOutput=False)
    out = nc.declare_dram_parameter("out", [p_pad, VS], F32, isOutput=True)

    m_tiles = p_pad // 128
    sum_len = int(sum(n_t))

    from contextlib import ExitStack

    with tile.TileContext(nc) as tc:
        stack = ExitStack()
        with stack:
            const = stack.enter_context(tc.tile_pool(name="const", bufs=1))
            res = stack.enter_context(tc.tile_pool(name="resident", bufs=1))
            state = stack.enter_context(tc.tile_pool(name="state", bufs=1))
            gxp = stack.enter_context(tc.tile_pool(name="gxb", bufs=1))
            gates = stack.enter_context(tc.tile_pool(name="gates", bufs=1))
            stmp = stack.enter_context(tc.tile_pool(name="scan_tmp", bufs=1))
            work = stack.enter_context(tc.tile_pool(name="work", bufs=3))
            psG = stack.enter_context(tc.tile_pool(name="psG", bufs=6, space="PSUM"))
            psT = stack.enter_context(tc.tile_pool(name="psT", bufs=2, space="PSUM"))
            dram = stack.enter_context(tc.tile_pool(name="dram", bufs=1, space="DRAM"))

            # ---- dummy collective first: absorbs ncfw warmup ASAP
            d_in = dram.tile([1, 128], F32)
            d_out = dram.tile([NC, 128], F32)
            d_in_sb = const.tile([1, 128], F32)
            nc.vector.memset(d_in_sb[:], 0.0)
            nc.sync.dma_start(out=d_in[:], in_=d_in_sb[:])
            nc.gpsimd.collective_compute(
                "AllGather",
                mybir.AluOpType.bypass,
                replica_groups=[list(range(NC))],
                ins=[d_in[:].opt()],
                outs=[d_out[:].opt()],
            )

            # ---- resident weights needed from t=0 (wres loads later, at t==3)
            whh_sb = res.tile([128, 8, G4], F16)
            nc.sync.dma_start(out=whh_sb[:], in_=whh.ap().rearrange("(k p) n -> p k n", p=128))
            bias_bc = res.tile([128, VS], F16)
            nc.sync.dma_start(
                out=bias_bc[:],
                in_=bass.AP(tensor=blin.ap().tensor, offset=0, ap=[[0, 128], [1, VS]]),
            )

            ident = const.tile([128, 128], F16)
            make_identity(nc, ident)
            idx_sb = const.tile([B, NGATHER], mybir.dt.int32)
            nc.sync.dma_start(out=idx_sb[:], in_=idx_in[:, :])

            # ---- scan state (allocated below the phase pools)
            packed_sb = state.tile([128, 8, p_pad], F16)
            if p_pad > sum_len:  # pad region never written: memset for sim
                nc.vector.memset(packed_sb[:, :, sum_len:p_pad], 0.0)
            hT = state.tile([128, 8, 128], F16)
            nc.vector.memset(hT[:], 0.0)
            c_st = state.tile([B, H], F32)
            nc.vector.memset(c_st[:], 0.0)

            # ---- phase A pools (released at t==3)
            phase_stack = ExitStack()
            pha = phase_stack.enter_context(tc.tile_pool(name="phase_a", bufs=1))
            wstr = phase_stack.enter_context(tc.tile_pool(name="wstream", bufs=1))
            gath = phase_stack.enter_context(tc.tile_pool(name="gath", bufs=3))

            tags_sb = pha.tile([128, 5, B], F16)
            nc.sync.dma_start(out=tags_sb[:], in_=tags_t.ap().rearrange("(k p) b -> p k b", p=128))
            wihx_sb = pha.tile([128, 4, G4], F16)
            nc.sync.dma_start(out=wihx_sb[:], in_=wihx.ap().rearrange("(k p) n -> p k n", p=128))

            # embedding gathers for all 8 cols
            gtiles = []
            for j in range(NGATHER):
                g = gath.tile([B, E], F16, tag="gather")
                nc.gpsimd.indirect_dma_start(
                    out=g[:],
                    out_offset=None,
                    in_=emb_tab[:],
                    in_offset=bass.IndirectOffsetOnAxis(ap=idx_sb[:, j : j + 1], axis=0),
                )
                gtiles.append(g)

            # tb = tags@Wiht^T + b  (bias row folded into wiht/tags)
            tb_sb = pha.tile([B, G4], F16)
            for n in range(8):
                w = wstr.tile([128, 5, 512], F16, tag="wstream")
                nc.sync.dma_start(
                    out=w[:],
                    in_=wiht.ap()[:, ts(n, 512)].rearrange("(k p) n -> p k n", p=128),
                )
                ps = psG.tile([128, 512], F32, space="PSUM")
                for k in range(5):
                    nc.tensor.matmul(
                        out=ps[:B, :],
                        lhsT=tags_sb[:, k, :],
                        rhs=w[:, k, :],
                        start=(k == 0),
                        stop=(k == 4),
                    )
                nc.vector.tensor_copy(out=tb_sb[:, ts(n, 512)], in_=ps[:B, :])

            # x^T for all 8 gathered cols (consumes gather tiles in order)
            xT = pha.tile([128, NGATHER, 4, 128], F16)
            for j in range(NGATHER):
                for k in range(4):
                    pt = psT.tile([128, 128], F16, space="PSUM")
                    nc.tensor.transpose(pt[:], gtiles[j][:, ts(k, 128)], ident[:])
                    nc.vector.tensor_copy(out=xT[:, j, k, :], in_=pt[:])

            # DRAM staging for gx
            gxl_dram = dram.tile([4, B, G4], F16, name="gx_local")
            ag_in = [dram.tile([B, G4], F16, name=f"ag_in{t_}") for t_ in range(4)]
            ag_out = [dram.tile([NC, B, G4], F16, name=f"ag_out{t_}") for t_ in range(4)]

            def emit_gx(col, dst_dram):
                # gx[col] = x[col]^T @ Wihx + tb  -> dst_dram [B, G4] fp16
                for n in range(8):
                    ps = psG.tile([128, 512], F32, space="PSUM")
                    for k in range(4):
                        nc.tensor.matmul(
                            out=ps[:B, :],
                            lhsT=xT[:, col, k, :],
                            rhs=wihx_sb[:, k, ts(n, 512)],
                            start=(k == 0),
                            stop=(k == 3),
                        )
                    gblk = work.tile([B, 512], F16, tag="gxout")
                    nc.vector.tensor_add(out=gblk[:], in0=ps[:B, :], in1=tb_sb[:, ts(n, 512)])
                    nc.sync.dma_start(out=dst_dram[:, ts(n, 512)], in_=gblk[:])

            def emit_ag(tau):
                nc.gpsimd.collective_compute(
                    "AllGather",
                    mybir.AluOpType.bypass,
                    replica_groups=[list(range(NC))],
                    ins=[ag_in[tau][:].opt()],
                    outs=[ag_out[tau][:].opt()],
                )

            # head: local step 0 and owned AG0 step; AG0 fires early
            emit_gx(0, gxl_dram[0])
            emit_gx(4, ag_in[0])
            emit_ag(0)
            # (col, dst) pairs interleaved into scan steps 0..2
            gx_fill = [
                (1, gxl_dram[1], None),
                (5, ag_in[1], 1),
                (2, gxl_dram[2], None),
                (6, ag_in[2], 2),
                (3, gxl_dram[3], None),
                (7, ag_in[3], 3),
            ]

            def emit_probes():
                # probe collectives: the CC stream is idle from ~350us on;
                # these measure pair-AG / AllToAll costs for free (outputs unused)
                pb1 = dram.tile([2, B, G4], F16, name="pb1")
                nc.gpsimd.collective_compute(
                    "AllGather", mybir.AluOpType.bypass,
                    replica_groups=[[0, 1], [2, 3], [4, 5], [6, 7]],
                    ins=[ag_in[0][:].opt()], outs=[pb1[:].opt()],
                )
                pb2 = dram.tile([2, B, G4], F16, name="pb2")
                nc.gpsimd.collective_compute(
                    "AllGather", mybir.AluOpType.bypass,
                    replica_groups=[[0, 4], [1, 5], [2, 6], [3, 7]],
                    ins=[ag_in[0][:].opt()], outs=[pb2[:].opt()],
                )
                pb3 = dram.tile([B, G4], F16, name="pb3")
                nc.gpsimd.collective_compute(
                    "AllToAll", mybir.AluOpType.bypass,
                    replica_groups=[list(range(NC))],
                    ins=[ag_in[0][:].opt()], outs=[pb3[:].opt()],
                )
                pb4 = dram.tile([2, 16, G4], F16, name="pb4")
                nc.gpsimd.collective_compute(
                    "AllGather", mybir.AluOpType.bypass,
                    replica_groups=[[0, 1], [2, 3], [4, 5], [6, 7]],
                    ins=[ag_in[0][:16, :].opt()], outs=[pb4[:].opt()],
                )
                pb5 = dram.tile([NC, 16, G4], F16, name="pb5")
                nc.gpsimd.collective_compute(
                    "AllGather", mybir.AluOpType.bypass,
                    replica_groups=[list(range(NC))],
                    ins=[ag_in[0][:16, :].opt()], outs=[pb5[:].opt()],
                )

            # projection emission machinery: unit = (m, nchunk), 8 matmuls each
            proj_units = [(m, n) for m in range(m_tiles) for n in range(NCH)]
            emitted = [0]
            late = {}  # wres / ostage pool, created at t==3

            def emit_proj_units(avail_rows, count):
                for _ in range(count):
                    if emitted[0] >= len(proj_units):
                        return
                    m, n = proj_units[emitted[0]]
                    if (m + 1) * 128 > avail_rows:
                        return
                    emitted[0] += 1
                    n0 = n * 512
                    nsz = min(512, VS - n0)
                    ps = psG.tile([128, 512], F32, space="PSUM")
                    for k in range(8):
                        nc.tensor.matmul(
                            out=ps[:, :nsz],
                            lhsT=packed_sb[:, k, ts(m, 128)],
                            rhs=late["wres"][:, k, n0 : n0 + nsz],
                            start=(k == 0),
                            stop=(k == 7),
                        )
                    ost = late["ostage"].tile([128, 512], F32, tag="ost")
                    nc.vector.tensor_add(
                        out=ost[:, :nsz], in0=ps[:, :nsz], in1=bias_bc[:, n0 : n0 + nsz]
                    )
                    nc.sync.dma_start(out=out[ts(m, 128), n0 : n0 + nsz], in_=ost[:, :nsz])

            def fetch_gxb(t):
                gxb = gxp.tile([B, G4], F16, tag="gxblk")
                if t < 4:
                    src = gxl_dram[t]
                else:
                    src = ag_out[(t - 4) // 8][(t - 4) % 8]
                nc.sync.dma_start(out=gxb[:], in_=src[:, :])
                return gxb

            gxb_cur = fetch_gxb(0)

            for t in range(T):
                if t == 3:
                    # phase A SBUF freed; load the projection weights there
                    phase_stack.close()
                    scan_pool = stack.enter_context(tc.tile_pool(name="scan_late", bufs=1))
                    wres = scan_pool.tile([128, 8, VS], F16)
                    nc.sync.dma_start(
                        out=wres[:], in_=wlin.ap().rearrange("(k p) n -> p k n", p=128)
                    )
                    late["wres"] = wres
                    late["ostage"] = stack.enter_context(
                        tc.tile_pool(name="ostage", bufs=2)
                    )

                gxb = gxb_cur

                # gate matmuls: per-gate consecutive k-accumulation; each gate
                # drains+activates immediately so the cell tail starts early
                gt = {}
                h = stmp.tile([B, H], F16, tag="h")
                for hf in range(2):
                    for n in range(4):
                        ps = psG.tile([128, 512], F32, space="PSUM", name="ps")
                        c0 = n * 1024 + hf * 512
                        for k in range(8):
                            nc.tensor.matmul(
                                out=ps[:B, :],
                                lhsT=hT[:, k, :],
                                rhs=whh_sb[:, k, c0 : c0 + 512],
                                start=(k == 0),
                                stop=(k == 7),
                            )
                        g = gates.tile([B, 512], F16, tag=f"gate{n}_{hf}")
                        gt[(n, hf)] = g
                        nc.vector.tensor_add(
                            out=g[:], in0=ps[:B, :], in1=gxb[:, c0 : c0 + 512]
                        )
                        nc.scalar.activation(g[:], g[:], GATE_FN[n])
                    # cell tail for this half on gpsimd/ACT (overlaps TensorE)
                    sl = slice(hf * 512, (hf + 1) * 512)
                    ig = stmp.tile([B, 512], F32, tag="ig")
                    nc.gpsimd.tensor_mul(out=ig[:], in0=gt[(0, hf)][:], in1=gt[(2, hf)][:])
                    fc = stmp.tile([B, 512], F32, tag="fc")
                    nc.gpsimd.tensor_mul(out=fc[:], in0=gt[(1, hf)][:], in1=c_st[:, sl])
                    nc.gpsimd.tensor_add(out=c_st[:, sl], in0=ig[:], in1=fc[:])
                    thc = stmp.tile([B, 512], F16, tag="thc")
                    nc.scalar.activation(thc[:], c_st[:, sl], AF.Tanh)
                    nc.gpsimd.tensor_mul(out=h[:, sl], in0=gt[(3, hf)][:], in1=thc[:])

                # transposes per half, with TensorE filler between groups for
                # tail-latency cover: gx cols during steps 0-2, proj units after
                for hf in range(2):
                    for k in range(4 * hf, 4 * hf + 4):
                        pt = psT.tile([128, 128], F16, space="PSUM")
                        nc.tensor.transpose(pt[:], h[:, ts(k, 128)], ident[:])
                        nc.vector.tensor_copy(out=hT[:, k, :], in_=pt[:])
                        if n_t[t] > 0:
                            nc.vector.tensor_copy(
                                out=packed_sb[:, k, off_t[t] : off_t[t] + n_t[t]],
                                in_=pt[:, : n_t[t]],
                            )
                    if t < 3:
                        col, dst, tau = gx_fill[2 * t + hf]
                        emit_gx(col, dst)
                        if tau is not None:
                            emit_ag(tau)
                            if tau == 3:
                                emit_probes()
                    else:
                        emit_proj_units(off_t[t], 2)
                if t + 1 < T:
                    gxb_cur = fetch_gxb(t + 1)

            # ---- projection epilogue
            emit_proj_units(p_pad, len(proj_units))

    nc.finalize()
    return nc


def kernel(features, tags, captions, lengths, W_embed, W_ih, W_hh, b_ih, b_hh, W_lin, b_lin):
    features = np.asarray(features, dtype=np.float32)
    tags = np.asarray(tags, dtype=np.float32)
    captions = np.asarray(captions)
    lengths = np.asarray(lengths)
    W_embed = np.asarray(W_embed, dtype=np.float32)
    W_ih = np.asarray(W_ih, dtype=np.float32)
    W_hh = np.asarray(W_hh, dtype=np.float32)
    b_ih = np.asarray(b_ih, dtype=np.float32)
    b_hh = np.asarray(b_hh, dtype=np.float32)
    W_lin = np.asarray(W_lin, dtype=np.float32)
    b_lin = np.asarray(b_lin, dtype=np.float32)

    # packing schedule (replicates reference pack_padded_sequence exactly)
    n_t = [int((lengths > t).sum()) for t in range(T)]
    off_t = np.concatenate([[0], np.cumsum(n_t)]).astype(np.int64)
    sum_len = int(off_t[-1])
    p_pad = ((sum_len + 127) // 128) * 128

    nc = _build(n_t, off_t, p_pad)

    emb_tab = np.concatenate([W_embed, features], axis=0).astype(np.float16)
    wihx = np.ascontiguousarray(W_ih[:, :E].T).astype(np.float16)
    wiht = np.zeros((5 * 128, G4), np.float16)
    wiht[:TAG] = W_ih[:, E:].T.astype(np.float16)
    wiht[TAG] = (b_ih + b_hh).astype(np.float16)
    tags_t = np.zeros((5 * 128, B), np.float16)
    tags_t[:TAG] = tags.T.astype(np.float16)
    tags_t[TAG] = 1.0
    whh = np.ascontiguousarray(W_hh.T).astype(np.float16)

    in_maps = []
    for c in range(NC):
        # gather cols: steps 0,1,2,3 then owned AG steps {4+c, 12+c, 20+c, 28+(c%4)}
        steps = [0, 1, 2, 3, 4 + c, 12 + c, 20 + c, 28 + (c % 4)]
        idx = np.empty((B, NGATHER), np.int32)
        for j, s in enumerate(steps):
            if s == 0:
                idx[:, j] = V + np.arange(B)
            else:
                idx[:, j] = captions[:, s - 1].astype(np.int32)
        wlin_c = np.ascontiguousarray(W_lin[c * VS : (c + 1) * VS].T).astype(np.float16)
        blin_c = np.ascontiguousarray(b_lin[c * VS : (c + 1) * VS]).astype(np.float16).reshape(1, VS)
        in_maps.append(
            {
                "emb_tab": emb_tab,
                "idx": idx,
                "wihx": wihx,
                "wiht": wiht,
                "tags_t": tags_t,
                "whh": whh,
                "wlin": wlin_c,
                "blin": blin_c,
            }
        )

    res = run_bass_kernel_spmd(nc, in_maps, list(range(NC)))

    out = np.empty((sum_len, V), np.float32)
    for c in range(NC):
        out[:, c * VS : (c + 1) * VS] = res.results[c]["out"][:sum_len]
    return out
